# revision 5
# baseline (speedup 1.0000x reference)
"""Trainium2 Bass kernel for nn_CrossAttention (B=4, Sq=Skv=4096, E=1024, H=512).

Sharding: 8 cores = batch(4) x Sq-halves(2). Each core computes its full
[2048, 1024] output block independently (no collectives).

Per-core pipeline (all matmuls bf16 x bf16 -> f32 PSUM):
  - inputs are cast f32->bf16 into DRAM scratch (SWDGE cast DMA), then
    DMA-transposed (xbar) into SBUF as qT/kvT [feature, token] layout.
  - K/V projections from kvT; Q projection (scaled by 1/sqrt(H)) from qT.
  - scores computed transposed: ST[k, q] = KT^T-slices @ QT, softmax without
    max-subtraction: exp on ACT (PSUM->SBUF, cast bf16), denominator via
    ones-vector matmul on PE, PV accumulated over k tiles in PSUM.
  - normalize with reciprocal broadcast (PE ones-broadcast trick), Wo + bias
    + residual (qT), Wfc back to natural [q, f] layout, LayerNorm via
    bn_stats/bn_aggr, gamma/beta, DMA out f32.
"""

import numpy as np


def _ensure_concourse():
    try:
        import concourse.bass  # noqa: F401
    except ImportError:
        import sys

        for p in ("/opt/trn_rl_repo", "/root/.axon_site/_ro/trn_rl_repo"):
            if p not in sys.path:
                sys.path.append(p)


_ensure_concourse()

from contextlib import ExitStack  # noqa: E402

import concourse.bacc as bacc  # noqa: E402
import concourse.bass as bass  # noqa: F401, E402
import concourse.mybir as mybir  # noqa: E402
import concourse.tile as tile  # noqa: E402
from concourse import bass_utils  # noqa: E402

P = 128
E = 1024
EI = E // P  # 8
H = 512
HI = H // P  # 4
SQ = 2048  # q rows per core (Sq / 2)
SKV = 4096
QB = 512  # q block (moving free dim)
NQB = SQ // QB  # 4
NKT = SKV // P  # 32
KVC = 2048  # kv chunk (token rows) for transposed staging
SCALE = 1.0 / float(np.sqrt(512.0))

f32 = mybir.dt.float32
bf16 = mybir.dt.bfloat16
AF = mybir.ActivationFunctionType
ALU = mybir.AluOpType

_cached_nc = None


def _build():
    global _cached_nc
    if _cached_nc is not None:
        return _cached_nc

    nc = bacc.Bacc("TRN2")

    q_d = nc.dram_tensor("q_loc", [SQ, E], f32, kind="ExternalInput").ap()
    kv_d = nc.dram_tensor("kv_loc", [SKV, E], f32, kind="ExternalInput").ap()
    wq_d = nc.dram_tensor("Wq", [E, H], f32, kind="ExternalInput").ap()
    wk_d = nc.dram_tensor("Wk", [E, H], f32, kind="ExternalInput").ap()
    wv_d = nc.dram_tensor("Wv", [E, H], f32, kind="ExternalInput").ap()
    wo_d = nc.dram_tensor("Wo", [H, E], f32, kind="ExternalInput").ap()
    bo_d = nc.dram_tensor("bo", [E], f32, kind="ExternalInput").ap()
    wfc_d = nc.dram_tensor("Wfc", [E, E], f32, kind="ExternalInput").ap()
    g_d = nc.dram_tensor("ln_gamma", [E], f32, kind="ExternalInput").ap()
    b_d = nc.dram_tensor("ln_beta", [E], f32, kind="ExternalInput").ap()
    out_d = nc.dram_tensor("out_loc", [SQ, E], f32, kind="ExternalOutput").ap()

    with tile.TileContext(nc) as tc, ExitStack() as ctx:
        const = ctx.enter_context(tc.tile_pool(name="const", bufs=1))
        psum = ctx.enter_context(tc.tile_pool(name="psum", bufs=2, space="PSUM"))
        dram = ctx.enter_context(tc.tile_pool(name="dram", bufs=1, space="DRAM"))

        # ---------- persistent SBUF ----------
        wq_sb = const.tile([P, EI, H], bf16, name="wq_sb")
        wo_sb = const.tile([P, HI, E], bf16, name="wo_sb")
        wfc_sb = const.tile([P, EI, E], bf16, name="wfc_sb")
        bo_sb = const.tile([P, EI], f32, name="bo_sb")
        g128 = const.tile([P, E], f32, name="g128")
        b128 = const.tile([P, E], f32, name="b128")
        ones_bf = const.tile([P, 1], bf16, name="ones_bf")
        eps_sb = const.tile([P, 1], f32, name="eps_sb")
        ones_f32 = const.tile([1, P], f32, name="ones_f32")
        kt_sb = const.tile([P, HI, SKV], bf16, name="kt_sb")  # KT [h, k]
        v_sb = const.tile([P, NKT, H], bf16, name="v_sb")  # V  [k, h]

        nc.vector.memset(ones_bf, 1.0)
        nc.vector.memset(eps_sb, 1e-5)
        nc.vector.memset(ones_f32, 1.0)

        # weights (cast f32 -> bf16 during SWDGE DMA)
        nc.gpsimd.dma_start(wq_sb[:], wq_d.rearrange("(ei p) h -> p ei h", p=P))
        nc.gpsimd.dma_start(wo_sb[:], wo_d.rearrange("(hj p) e -> p hj e", p=P))
        nc.gpsimd.dma_start(wfc_sb[:], wfc_d.rearrange("(ej p) f -> p ej f", p=P))
        nc.sync.dma_start(bo_sb[:], bo_d.rearrange("(ej p) -> p ej", p=P))
        nc.gpsimd.dma_start(g128[:], g_d.rearrange("(a f) -> a f", a=1).broadcast_to((P, E)))
        nc.gpsimd.dma_start(b128[:], b_d.rearrange("(a f) -> a f", a=1).broadcast_to((P, E)))

        # bf16 copies of the activations in DRAM (for xbar DMA-transpose)
        kv_bf = dram.tile([SKV, E], bf16, name="kv_bf")
        q_bf = dram.tile([SQ, E], bf16, name="q_bf")
        for c in range(SKV // KVC):
            nc.gpsimd.dma_start(
                kv_bf[c * KVC : (c + 1) * KVC, :], kv_d[c * KVC : (c + 1) * KVC, :]
            )
        for c in range(NQB):
            nc.gpsimd.dma_start(
                q_bf[c * QB : (c + 1) * QB, :], q_d[c * QB : (c + 1) * QB, :]
            )

        # ---------- phase 1: K / V projections ----------
        with tc.tile_pool(name="p1", bufs=1) as p1:
            wk_sb = p1.tile([P, EI, H], bf16, name="wk_sb")
            wv_sb = p1.tile([P, EI, H], bf16, name="wv_sb")
            nc.gpsimd.dma_start(wk_sb[:], wk_d.rearrange("(ei p) h -> p ei h", p=P))
            nc.gpsimd.dma_start(wv_sb[:], wv_d.rearrange("(ei p) h -> p ei h", p=P))

            for c in range(SKV // KVC):
                kvt = p1.tile([P, EI, KVC], bf16, name="kvt", tag="kvt", bufs=2)
                for ei in range(EI):
                    nc.sync.dma_start(
                        kvt[:, ei : ei + 1, :],
                        kv_bf[c * KVC : (c + 1) * KVC, ei * P : (ei + 1) * P],
                        transpose=True,
                    )
                # KT[h, k] += Wk[e,h]^T kvT[e,k]
                for hi in range(HI):
                    for kc in range(KVC // QB):
                        pk = psum.tile([P, QB], f32, name="pk", tag="a", bufs=2)
                        for ei in range(EI):
                            nc.tensor.matmul(
                                pk,
                                wk_sb[:, ei : ei + 1, hi * P : (hi + 1) * P],
                                kvt[:, ei : ei + 1, kc * QB : (kc + 1) * QB],
                                start=(ei == 0),
                                stop=(ei == EI - 1),
                            )
                        o = c * KVC + kc * QB
                        nc.scalar.copy(kt_sb[:, hi : hi + 1, o : o + QB], pk)
                # V[k, h] += kvT[e,k]^T Wv[e,h]
                for kt in range(KVC // P):
                    pv = psum.tile([P, H], f32, name="pv", tag="a", bufs=2)
                    for ei in range(EI):
                        nc.tensor.matmul(
                            pv,
                            kvt[:, ei : ei + 1, kt * P : (kt + 1) * P],
                            wv_sb[:, ei : ei + 1, :],
                            start=(ei == 0),
                            stop=(ei == EI - 1),
                        )
                    g = c * (KVC // P) + kt
                    nc.scalar.copy(v_sb[:, g : g + 1, :], pv)

        # ---------- phase 2: attention + output per q block ----------
        with tc.tile_pool(name="p2", bufs=2) as p2:
            for qb in range(NQB):
                qt = p2.tile([P, EI, QB], bf16, name="qt", tag="qt", bufs=2)
                for ei in range(EI):
                    nc.sync.dma_start(
                        qt[:, ei : ei + 1, :],
                        q_bf[qb * QB : (qb + 1) * QB, ei * P : (ei + 1) * P],
                        transpose=True,
                    )
                # QT[h, q] (scaled by 1/sqrt(H))
                qts = []
                for hi in range(HI):
                    pq = psum.tile([P, QB], f32, name="pq", tag="a", bufs=2)
                    for ei in range(EI):
                        nc.tensor.matmul(
                            pq,
                            wq_sb[:, ei : ei + 1, hi * P : (hi + 1) * P],
                            qt[:, ei : ei + 1, :],
                            start=(ei == 0),
                            stop=(ei == EI - 1),
                        )
                    qs = p2.tile([P, QB], bf16, name="qs", tag="qts", bufs=8)
                    nc.scalar.mul(qs, pq, SCALE)
                    qts.append(qs)

                # attention: ST[k,q] -> exp -> PV, denominator via ones matmul
                ctx_ps = [
                    psum.tile([P, QB], f32, name=f"cx{hj}", tag="ctx", bufs=4)
                    for hj in range(HI)
                ]
                den = psum.tile([1, QB], f32, name="den", tag="misc", bufs=2)
                for kt in range(NKT):
                    st = psum.tile([P, QB], f32, name="st", tag="a", bufs=2)
                    for hi in range(HI):
                        nc.tensor.matmul(
                            st,
                            kt_sb[:, hi : hi + 1, kt * P : (kt + 1) * P],
                            qts[hi],
                            start=(hi == 0),
                            stop=(hi == HI - 1),
                        )
                    ex = p2.tile([P, QB], bf16, name="ex", tag="ex", bufs=4)
                    nc.scalar.activation(ex, st, AF.Exp)
                    for hj in range(HI):
                        nc.tensor.matmul(
                            ctx_ps[hj],
                            v_sb[:, kt : kt + 1, hj * P : (hj + 1) * P],
                            ex,
                            start=(kt == 0),
                            stop=(kt == NKT - 1),
                        )
                    nc.tensor.matmul(
                        den, ones_bf[:], ex, start=(kt == 0), stop=(kt == NKT - 1)
                    )

                # normalize: ctxb = ctx * (1/den) broadcast over partitions
                rec1 = p2.tile([1, QB], f32, name="rec1", tag="rec1", bufs=2)
                nc.vector.reciprocal(rec1, den)
                rps = psum.tile([P, QB], f32, name="rps", tag="a", bufs=2)
                nc.tensor.matmul(rps, ones_f32[:], rec1)
                rec128 = p2.tile([P, QB], f32, name="rec128", tag="rec128", bufs=2)
                nc.scalar.copy(rec128, rps)
                ctxb = []
                for hj in range(HI):
                    cb = p2.tile([P, QB], bf16, name="cb", tag="ctxb", bufs=8)
                    nc.vector.tensor_tensor(cb, ctx_ps[hj], rec128, op=ALU.mult)
                    ctxb.append(cb)

                # Wo + bias + residual -> y^T [e, q] (bf16)
                ys = []
                for ej in range(EI):
                    po = psum.tile([P, QB], f32, name="po", tag="a", bufs=2)
                    for hj in range(HI):
                        nc.tensor.matmul(
                            po,
                            wo_sb[:, hj : hj + 1, ej * P : (ej + 1) * P],
                            ctxb[hj],
                            start=(hj == 0),
                            stop=(hj == HI - 1),
                        )
                    y1 = p2.tile([P, QB], bf16, name="y1", tag="y1", bufs=3)
                    nc.scalar.add(y1, po, bo_sb[:, ej : ej + 1])
                    y = p2.tile([P, QB], bf16, name="y", tag="y", bufs=16)
                    nc.vector.tensor_tensor(y, y1, qt[:, ej : ej + 1, :], op=ALU.add)
                    ys.append(y)

                # Wfc back to natural [q, f], then LayerNorm + out
                for qi in range(QB // P):
                    o2 = p2.tile([P, E], f32, name="o2", tag="o2", bufs=2)
                    for fj in range(2):
                        pf = psum.tile([P, H], f32, name="pf", tag="a", bufs=2)
                        for ej in range(EI):
                            nc.tensor.matmul(
                                pf,
                                ys[ej][:, qi * P : (qi + 1) * P],
                                wfc_sb[:, ej : ej + 1, fj * H : (fj + 1) * H],
                                start=(ej == 0),
                                stop=(ej == EI - 1),
                            )
                        nc.vector.tensor_copy(o2[:, fj * H : (fj + 1) * H], pf)
                    st6 = p2.tile([P, 2, 6], f32, name="st6", tag="st6", bufs=3)
                    for g in range(2):
                        nc.vector.bn_stats(
                            st6[:, g : g + 1, :], o2[:, g * H : (g + 1) * H]
                        )
                    st2 = p2.tile([P, 2], f32, name="st2", tag="st2", bufs=3)
                    nc.vector.bn_aggr(st2, st6.rearrange("p a b -> p (a b)"))
                    stdt = p2.tile([P, 1], f32, name="stdt", tag="stdt", bufs=3)
                    nc.scalar.activation(stdt, st2[:, 1:2], AF.Sqrt, bias=eps_sb)
                    rstd = p2.tile([P, 1], f32, name="rstd", tag="rstd", bufs=3)
                    nc.vector.reciprocal(rstd, stdt)
                    nmr = p2.tile([P, 1], f32, name="nmr", tag="nmr", bufs=3)
                    nc.vector.tensor_tensor(nmr, st2[:, 0:1], rstd, op=ALU.mult)
                    nc.vector.tensor_scalar_mul(nmr, nmr, -1.0)
                    nrm = p2.tile([P, E], f32, name="nrm", tag="nrm", bufs=2)
                    nc.scalar.activation(nrm, o2, AF.Identity, bias=nmr, scale=rstd)
                    outt = p2.tile([P, E], f32, name="outt", tag="outt", bufs=2)
                    nc.vector.tensor_tensor(outt, nrm, g128, op=ALU.mult)
                    nc.vector.tensor_tensor(outt, outt, b128, op=ALU.add)
                    r0 = qb * QB + qi * P
                    nc.sync.dma_start(out_d[r0 : r0 + P, :], outt)

    nc.compile()
    _cached_nc = nc
    return nc


def _in_maps(q_feat, kv_feat, Wq, Wk, Wv, Wo, bo, Wfc, ln_gamma, ln_beta):
    maps = []
    for c in range(8):
        b, half = c // 2, c % 2
        maps.append(
            {
                "q_loc": np.ascontiguousarray(
                    q_feat[b, half * SQ : (half + 1) * SQ], dtype=np.float32
                ),
                "kv_loc": np.ascontiguousarray(kv_feat[b], dtype=np.float32),
                "Wq": np.asarray(Wq, np.float32),
                "Wk": np.asarray(Wk, np.float32),
                "Wv": np.asarray(Wv, np.float32),
                "Wo": np.asarray(Wo, np.float32),
                "bo": np.asarray(bo, np.float32),
                "Wfc": np.asarray(Wfc, np.float32),
                "ln_gamma": np.asarray(ln_gamma, np.float32),
                "ln_beta": np.asarray(ln_beta, np.float32),
            }
        )
    return maps


def run_spmd(inputs, **kwargs):
    """Run the SPMD kernel; returns (full_output, BassKernelResults)."""
    nc = _build()
    maps = _in_maps(**inputs)
    res = bass_utils.run_bass_kernel_spmd(nc, maps, core_ids=list(range(8)), **kwargs)
    out = np.empty((4, 2 * SQ, E), np.float32)
    for c in range(8):
        b, half = c // 2, c % 2
        out[b, half * SQ : (half + 1) * SQ] = res.results[c]["out_loc"]
    return out, res


def kernel(**inputs):
    out, _ = run_spmd(inputs)
    return out


# revision 7
# speedup vs baseline: 161.3799x; 161.3799x over previous
"""Trainium2 Bass kernel for nn_CrossAttention (B=4, Sq=Skv=4096, E=1024, H=512).

Sharding: 8 cores = batch(4) x Sq-halves(2). Each core computes its full
[2048, 1024] output block independently (no collectives).

Per-core pipeline (all matmuls bf16 x bf16 -> f32 PSUM):
  - inputs are cast f32->bf16 into DRAM scratch (SWDGE cast DMA), then
    DMA-transposed (xbar) into SBUF as qT/kvT [feature, token] layout.
  - K/V projections from kvT; Q projection (scaled by 1/sqrt(H)) from qT.
  - scores computed transposed: ST[k, q] = KT-slices^T @ QT, softmax without
    max-subtraction: exp on ACT (PSUM->SBUF, cast bf16), denominator via
    ones-vector matmul on PE, PV accumulated over k tiles in PSUM.
  - normalize with reciprocal broadcast (PE ones-broadcast trick), Wo + bias
    + residual (qT), Wfc back to natural [q, f] layout, LayerNorm via
    bn_stats/bn_aggr, gamma/beta, DMA out f32.

The `repeat` build parameter traces the compute body R times (same data,
same outputs) inside one NEFF — used only for differential timing.
"""

import numpy as np


def _ensure_concourse():
    try:
        import concourse.bass  # noqa: F401
    except ImportError:
        import sys

        for p in ("/opt/trn_rl_repo", "/root/.axon_site/_ro/trn_rl_repo"):
            if p not in sys.path:
                sys.path.append(p)


_ensure_concourse()

from contextlib import ExitStack  # noqa: E402

import concourse.bacc as bacc  # noqa: E402
import concourse.mybir as mybir  # noqa: E402
import concourse.tile as tile  # noqa: E402
from concourse import bass_utils  # noqa: E402

P = 128
E = 1024
EI = E // P  # 8
H = 512
HI = H // P  # 4
SQ = 2048  # q rows per core (Sq / 2)
SKV = 4096
QB = 512  # q block (moving free dim)
NQB = SQ // QB  # 4
NKT = SKV // P  # 32
KVC = 2048  # kv chunk (token rows) for transposed staging
SCALE = 1.0 / float(np.sqrt(512.0))

f32 = mybir.dt.float32
bf16 = mybir.dt.bfloat16
AF = mybir.ActivationFunctionType
ALU = mybir.AluOpType

_cached_nc = {}


def _build(repeat=1):
    if repeat in _cached_nc:
        return _cached_nc[repeat]

    nc = bacc.Bacc("TRN2")

    q_d = nc.dram_tensor("q_loc", [SQ, E], f32, kind="ExternalInput").ap()
    kv_d = nc.dram_tensor("kv_loc", [SKV, E], f32, kind="ExternalInput").ap()
    wq_d = nc.dram_tensor("Wq", [E, H], f32, kind="ExternalInput").ap()
    wk_d = nc.dram_tensor("Wk", [E, H], f32, kind="ExternalInput").ap()
    wv_d = nc.dram_tensor("Wv", [E, H], f32, kind="ExternalInput").ap()
    wo_d = nc.dram_tensor("Wo", [H, E], f32, kind="ExternalInput").ap()
    bo_d = nc.dram_tensor("bo", [E], f32, kind="ExternalInput").ap()
    wfc_d = nc.dram_tensor("Wfc", [E, E], f32, kind="ExternalInput").ap()
    g_d = nc.dram_tensor("ln_gamma", [E], f32, kind="ExternalInput").ap()
    b_d = nc.dram_tensor("ln_beta", [E], f32, kind="ExternalInput").ap()
    out_d = nc.dram_tensor("out_loc", [SQ, E], f32, kind="ExternalOutput").ap()

    with tile.TileContext(nc) as tc, ExitStack() as ctx:
        const = ctx.enter_context(tc.tile_pool(name="const", bufs=1))
        psum = ctx.enter_context(tc.tile_pool(name="psum", bufs=2, space="PSUM"))
        dram = ctx.enter_context(tc.tile_pool(name="dram", bufs=1, space="DRAM"))

        # ---------- persistent SBUF ----------
        wq_sb = const.tile([P, EI, H], bf16, name="wq_sb")
        wo_sb = const.tile([P, HI, E], bf16, name="wo_sb")
        wfc_sb = const.tile([P, EI, E], bf16, name="wfc_sb")
        bo_sb = const.tile([P, EI], f32, name="bo_sb")
        g128 = const.tile([P, E], f32, name="g128")
        b128 = const.tile([P, E], f32, name="b128")
        ones_bf = const.tile([P, 1], bf16, name="ones_bf")
        eps_sb = const.tile([P, 1], f32, name="eps_sb")
        ones_f32 = const.tile([1, P], f32, name="ones_f32")
        kt_sb = const.tile([P, HI, SKV], bf16, name="kt_sb")  # KT [h, k]
        v_sb = const.tile([P, NKT, H], bf16, name="v_sb")  # V  [k, h]

        nc.vector.memset(ones_bf, 1.0)
        nc.vector.memset(eps_sb, 1e-5)
        nc.vector.memset(ones_f32, 1.0)

        # weights (cast f32 -> bf16 during SWDGE DMA)
        nc.gpsimd.dma_start(wq_sb[:], wq_d.rearrange("(ei p) h -> p ei h", p=P))
        nc.gpsimd.dma_start(wo_sb[:], wo_d.rearrange("(hj p) e -> p hj e", p=P))
        nc.gpsimd.dma_start(wfc_sb[:], wfc_d.rearrange("(ej p) f -> p ej f", p=P))
        nc.sync.dma_start(bo_sb[:], bo_d.rearrange("(ej p) -> p ej", p=P))
        nc.gpsimd.dma_start(
            g128[:], g_d.rearrange("(a f) -> a f", a=1).broadcast_to((P, E))
        )
        nc.gpsimd.dma_start(
            b128[:], b_d.rearrange("(a f) -> a f", a=1).broadcast_to((P, E))
        )

        # bf16 copies of the activations in DRAM (for xbar DMA-transpose)
        kv_bf = dram.tile([SKV, E], bf16, name="kv_bf")
        q_bf = dram.tile([SQ, E], bf16, name="q_bf")
        for c in range(SKV // KVC):
            nc.gpsimd.dma_start(
                kv_bf[c * KVC : (c + 1) * KVC, :], kv_d[c * KVC : (c + 1) * KVC, :]
            )
        for c in range(NQB):
            nc.gpsimd.dma_start(
                q_bf[c * QB : (c + 1) * QB, :], q_d[c * QB : (c + 1) * QB, :]
            )

        for _rep in range(repeat):
            # ---------- phase 1: K / V projections ----------
            with tc.tile_pool(name="p1", bufs=1) as p1:
                wk_sb = p1.tile([P, EI, H], bf16, name="wk_sb")
                wv_sb = p1.tile([P, EI, H], bf16, name="wv_sb")
                nc.gpsimd.dma_start(wk_sb[:], wk_d.rearrange("(ei p) h -> p ei h", p=P))
                nc.gpsimd.dma_start(wv_sb[:], wv_d.rearrange("(ei p) h -> p ei h", p=P))

                for c in range(SKV // KVC):
                    kvt = p1.tile([P, EI, KVC], bf16, name="kvt", tag="kvt", bufs=2)
                    for ei in range(EI):
                        nc.sync.dma_start(
                            kvt[:, ei : ei + 1, :],
                            kv_bf[c * KVC : (c + 1) * KVC, ei * P : (ei + 1) * P],
                            transpose=True,
                        )
                    # KT[h, k] += Wk[e,h]^T kvT[e,k]
                    for hi in range(HI):
                        for kc in range(KVC // QB):
                            pk = psum.tile([P, QB], f32, name="pk", tag="a", bufs=2)
                            for ei in range(EI):
                                nc.tensor.matmul(
                                    pk,
                                    wk_sb[:, ei : ei + 1, hi * P : (hi + 1) * P],
                                    kvt[:, ei : ei + 1, kc * QB : (kc + 1) * QB],
                                    start=(ei == 0),
                                    stop=(ei == EI - 1),
                                )
                            o = c * KVC + kc * QB
                            nc.scalar.copy(kt_sb[:, hi : hi + 1, o : o + QB], pk)
                    # V[k, h] += kvT[e,k]^T Wv[e,h]
                    for kt in range(KVC // P):
                        pv = psum.tile([P, H], f32, name="pv", tag="a", bufs=2)
                        for ei in range(EI):
                            nc.tensor.matmul(
                                pv,
                                kvt[:, ei : ei + 1, kt * P : (kt + 1) * P],
                                wv_sb[:, ei : ei + 1, :],
                                start=(ei == 0),
                                stop=(ei == EI - 1),
                            )
                        g = c * (KVC // P) + kt
                        nc.scalar.copy(v_sb[:, g : g + 1, :], pv)

            # ---------- phase 2: attention + output per q block ----------
            with tc.tile_pool(name="p2", bufs=2) as p2:
                for qb in range(NQB):
                    qt = p2.tile([P, EI, QB], bf16, name="qt", tag="qt", bufs=2)
                    for ei in range(EI):
                        nc.sync.dma_start(
                            qt[:, ei : ei + 1, :],
                            q_bf[qb * QB : (qb + 1) * QB, ei * P : (ei + 1) * P],
                            transpose=True,
                        )
                    # QT[h, q] (scaled by 1/sqrt(H))
                    qts = []
                    for hi in range(HI):
                        pq = psum.tile([P, QB], f32, name="pq", tag="a", bufs=2)
                        for ei in range(EI):
                            nc.tensor.matmul(
                                pq,
                                wq_sb[:, ei : ei + 1, hi * P : (hi + 1) * P],
                                qt[:, ei : ei + 1, :],
                                start=(ei == 0),
                                stop=(ei == EI - 1),
                            )
                        qs = p2.tile([P, QB], bf16, name="qs", tag="qts", bufs=8)
                        nc.scalar.mul(qs, pq, SCALE)
                        qts.append(qs)

                    # attention: ST[k,q] -> exp -> PV, denominator via ones
                    ctx_ps = [
                        psum.tile([P, QB], f32, name=f"cx{hj}", tag="ctx", bufs=4)
                        for hj in range(HI)
                    ]
                    den = psum.tile([1, QB], f32, name="den", tag="misc", bufs=2)
                    for kt in range(NKT):
                        st = psum.tile([P, QB], f32, name="st", tag="a", bufs=2)
                        for hi in range(HI):
                            nc.tensor.matmul(
                                st,
                                kt_sb[:, hi : hi + 1, kt * P : (kt + 1) * P],
                                qts[hi],
                                start=(hi == 0),
                                stop=(hi == HI - 1),
                            )
                        ex = p2.tile([P, QB], bf16, name="ex", tag="ex", bufs=4)
                        nc.scalar.activation(ex, st, AF.Exp)
                        for hj in range(HI):
                            nc.tensor.matmul(
                                ctx_ps[hj],
                                v_sb[:, kt : kt + 1, hj * P : (hj + 1) * P],
                                ex,
                                start=(kt == 0),
                                stop=(kt == NKT - 1),
                            )
                        nc.tensor.matmul(
                            den, ones_bf[:], ex, start=(kt == 0), stop=(kt == NKT - 1)
                        )

                    # normalize: ctxb = ctx * (1/den) broadcast over partitions
                    rec1 = p2.tile([1, QB], f32, name="rec1", tag="rec1", bufs=2)
                    nc.vector.reciprocal(rec1, den)
                    rps = psum.tile([P, QB], f32, name="rps", tag="a", bufs=2)
                    nc.tensor.matmul(rps, ones_f32[:], rec1)
                    rec128 = p2.tile([P, QB], f32, name="rec128", tag="rec128", bufs=2)
                    nc.scalar.copy(rec128, rps)
                    ctxb = []
                    for hj in range(HI):
                        cb = p2.tile([P, QB], bf16, name="cb", tag="ctxb", bufs=8)
                        nc.vector.tensor_tensor(cb, ctx_ps[hj], rec128, op=ALU.mult)
                        ctxb.append(cb)

                    # Wo + bias + residual -> y^T [e, q] (bf16)
                    ys = []
                    for ej in range(EI):
                        po = psum.tile([P, QB], f32, name="po", tag="a", bufs=2)
                        for hj in range(HI):
                            nc.tensor.matmul(
                                po,
                                wo_sb[:, hj : hj + 1, ej * P : (ej + 1) * P],
                                ctxb[hj],
                                start=(hj == 0),
                                stop=(hj == HI - 1),
                            )
                        y1 = p2.tile([P, QB], bf16, name="y1", tag="y1", bufs=3)
                        nc.scalar.add(y1, po, bo_sb[:, ej : ej + 1])
                        y = p2.tile([P, QB], bf16, name="y", tag="y", bufs=16)
                        nc.vector.tensor_tensor(y, y1, qt[:, ej : ej + 1, :], op=ALU.add)
                        ys.append(y)

                    # Wfc back to natural [q, f], then LayerNorm + out
                    for qi in range(QB // P):
                        o2 = p2.tile([P, E], f32, name="o2", tag="o2", bufs=2)
                        for fj in range(2):
                            pf = psum.tile([P, H], f32, name="pf", tag="a", bufs=2)
                            for ej in range(EI):
                                nc.tensor.matmul(
                                    pf,
                                    ys[ej][:, qi * P : (qi + 1) * P],
                                    wfc_sb[:, ej : ej + 1, fj * H : (fj + 1) * H],
                                    start=(ej == 0),
                                    stop=(ej == EI - 1),
                                )
                            nc.vector.tensor_copy(o2[:, fj * H : (fj + 1) * H], pf)
                        st6 = p2.tile([P, 2, 6], f32, name="st6", tag="st6", bufs=3)
                        for g in range(2):
                            nc.vector.bn_stats(
                                st6[:, g : g + 1, :], o2[:, g * H : (g + 1) * H]
                            )
                        st2 = p2.tile([P, 2], f32, name="st2", tag="st2", bufs=3)
                        nc.vector.bn_aggr(st2, st6.rearrange("p a b -> p (a b)"))
                        stdt = p2.tile([P, 1], f32, name="stdt", tag="stdt", bufs=3)
                        nc.scalar.activation(stdt, st2[:, 1:2], AF.Sqrt, bias=eps_sb)
                        rstd = p2.tile([P, 1], f32, name="rstd", tag="rstd", bufs=3)
                        nc.vector.reciprocal(rstd, stdt)
                        nmr = p2.tile([P, 1], f32, name="nmr", tag="nmr", bufs=3)
                        nc.vector.tensor_tensor(nmr, st2[:, 0:1], rstd, op=ALU.mult)
                        nc.vector.tensor_scalar_mul(nmr, nmr, -1.0)
                        nrm = p2.tile([P, E], f32, name="nrm", tag="nrm", bufs=2)
                        nc.scalar.activation(nrm, o2, AF.Identity, bias=nmr, scale=rstd)
                        outt = p2.tile([P, E], f32, name="outt", tag="outt", bufs=2)
                        nc.vector.tensor_tensor(outt, nrm, g128, op=ALU.mult)
                        nc.vector.tensor_tensor(outt, outt, b128, op=ALU.add)
                        r0 = qb * QB + qi * P
                        nc.sync.dma_start(out_d[r0 : r0 + P, :], outt)

    nc.compile()
    _cached_nc[repeat] = nc
    return nc


def _in_maps(q_feat, kv_feat, Wq, Wk, Wv, Wo, bo, Wfc, ln_gamma, ln_beta):
    maps = []
    for c in range(8):
        b, half = c // 2, c % 2
        maps.append(
            {
                "q_loc": np.ascontiguousarray(
                    q_feat[b, half * SQ : (half + 1) * SQ], dtype=np.float32
                ),
                "kv_loc": np.ascontiguousarray(kv_feat[b], dtype=np.float32),
                "Wq": np.asarray(Wq, np.float32),
                "Wk": np.asarray(Wk, np.float32),
                "Wv": np.asarray(Wv, np.float32),
                "Wo": np.asarray(Wo, np.float32),
                "bo": np.asarray(bo, np.float32),
                "Wfc": np.asarray(Wfc, np.float32),
                "ln_gamma": np.asarray(ln_gamma, np.float32),
                "ln_beta": np.asarray(ln_beta, np.float32),
            }
        )
    return maps


def run_spmd(inputs, repeat=1, **kwargs):
    """Run the SPMD kernel; returns (full_output, BassKernelResults)."""
    nc = _build(repeat)
    maps = _in_maps(**inputs)
    res = bass_utils.run_bass_kernel_spmd(nc, maps, core_ids=list(range(8)), **kwargs)
    out = np.empty((4, 2 * SQ, E), np.float32)
    for c in range(8):
        b, half = c // 2, c % 2
        out[b, half * SQ : (half + 1) * SQ] = res.results[c]["out_loc"]
    return out, res


def kernel(**inputs):
    out, _ = run_spmd(inputs)
    return out
